# revision 73
# baseline (speedup 1.0000x reference)
"""Trainium2 Bass kernel for nn_AiMAiPartiallyConnectedLayers.

26 independent MLPs (5 -> 64 -> 64 -> 1, tanh) applied per node type over a
batch of 65536 samples; output [B, 26] fp32.  Pure data parallel over 8
NeuronCores (8192 samples each); ~235.5-237 us HW time (cool device;
baseline was 257.8), rel err ~3.3e-3 (bf16 matmul precision).

Design (ScalarE/tanh is the bottleneck engine; its ~215us busy IS the
kernel -- everything else hides behind it):
  - Types in 13 pairs, packed block-diagonal [128, 128] bf16 per pair.
    Input pre-transposed host-side to xt[tile, q, 128, 512]; plane q holds
    pairs 4q+k on partition rows 32k:32k+11 (10 channel rows + a ones row
    folding b1).  Layer-1 matmuls: K=32 with row tile_position (32k, 0);
    the 4 matmuls of a plane co-issue on disjoint PE row-quadrants.
  - ACT1 is one op per PLANE over a 4-bank PSUM region [128, 4, 512]
    (2048 cols).  ps1 is a single 4-bank buffer; refills hide under the
    ACT2 drains between ACT1 ops.
  - L2 path: per (pair, tile) matmul into ps2 [128,512], then a DVE
    tensor_scalar_add stages the pre-activation to SBUF **fp16** WITH the
    b2 bias folded in (per-partition AP scalar broadcasts at copy cost).
    ACT2 is then a BIAS-FREE tanh from SBUF, so one op spans a 2-pair
    group x 2-tile super-tile (2048 cols) -- ~6.2us/tile of ACT2 vs 7.8
    for per-pair-per-tile PSUM ops.  Last 2 tiles use per-pair single-
    tile ACT2s so the tanh tail starts early (avoids +4us tail bunching).
  - Layer-3 weights for pair p have nonzero columns 2p, 2p+1 inside the
    pair block, so ALL 13 pairs accumulate into rows 0:26 of ONE PSUM bank
    per tile (start/stop flags).  One DVE copy moves [26, 512] to SBUF,
    one DMA writes a type-major DRAM output [26, bc]; the HOST transposes
    and adds b3.  L3s are deferred one slot after their ACT2 so the
    in-order PE queue never stalls on ScalarE.
  - Slot pipeline over (tile, plane): ACT1(q) | 1 ACT2 pop | L3s(lag-1) |
    2 stage drains | L1(next plane) | remaining drains | 1 ACT2 pop.
    Stage-drain FIFO keys as before; ACT2 groups become ready 2 slots
    after their last stage (DVE slack).
  - PSUM: ps1 4 banks (single) + ps2 2x1 + ps3 2x1 = 8 banks exactly.
  - Setup DMAs ordered by need across 3 queues (see comment in code);
    tile-0 ACT1s split in 2-pair halves to soften the cold-PE ramp.

Measured cost models (this session, from ntff profiles -- the v1
docstring's ACT model was WRONG):
  - ACTIVATE: dur ~= 0.846ns/col + 167ns fixed (512col=636, 1024=1114,
    2048=1970, 4096=3630).  Marginal dominates; ScalarE floor for all
    tanh cols is ~180us/core.  Op count only matters ~167ns at a time.
  - DVE COPY/TENSOR_SCALAR [128,512] fp32->fp16: ~690ns (~0.84ns/col).
  - bf16 MM N=512 ~213ns warm, ~600 cold; LDW 128x128 ~100ns.  fp32
    matmuls lower to TWO passes (fp32_mode=LOW_HIGH, ~1us each) -- never
    matmul fp32 casually.
  - HAM: PE clock-gate DEFAULTS to 4/8 (1.2GHz); only ~3.4us of sustained
    PE busy releases it, any ~3.4us idle window re-throttles.  This
    ScalarE-bound kernel has PE duty ~60-70% -> oscillates (~28-63us of
    matmuls run at half clock).  Dummy-matmul warming FAILS: each dummy
    costs 400-600ns of in-order PE queue (692us disaster at 860 dummies);
    engines can't even start before their iqueue DMA lands (~6-7.5us).
  - DMA: one hw DGE queue ~77-180GB/s; sync(SP) queue is a SOFTWARE DGE
    (slower).  dma_start allowed only on gpsimd/sync/scalar.  Each
    enqueue instruction costs ~600ns on the issuing engine; a consumer
    waits for the LAST byte of the whole DMA it depends on -> split
    setup weights by first-use and order by need.
  - Thermal: 4+ back-to-back HW runs downclock EVERYTHING ~20-25%
    (285us vs 239 for the identical NEFF).  Sleep >=2min before judging.

Tried and rejected this session:
  - HAM dummy matmuls (mid-kernel or long startup bursts): see above.
  - 4-pair ACT2 groups (4096 cols): ScalarE busy 206.7us (best seen!) but
    8-stage gating + 8-MM L3 bursts idle-bunch the PE -> throttle 101us,
    +23us ScalarE stalls -> 257us net.  2-pair groups are the sweet spot.
  - DVE rational tanh (reciprocal ~6ns/col) and polynomial tanh (~8 ops
    x ~0.85ns/col + 260ns/op fixed ~= 5-9ns/col effective): both lose;
    ScalarE does tanh at 0.85ns/col.
  - PSUM-only ACT2 merges (2-tile [128,2,512] ps2): blocked by the 8-bank
    wall (ps1 4 + ps2 2x2 + ps3 2 = 10) unless ACT1 drops to 2-pair
    groups, which costs more than the merge saves under the REAL ACT
    model (fixed is 167ns, not 420).
Second-round findings (3 more HW iterations, all reverted to this
config which measured 238.9/239.1/239.2 across independent cool runs):
  - Steady-state ScalarE gaps are ~565ns x2 per super-tile boundary
    (~7us total): when act2q runs dry there, ACT1 waits on the ps1
    refill chain L1 <- L2-drains <- DVE-ps2-frees.  Pop-budget leveling
    (1/slot + conditional), 1-slot vs 2-slot ACT2 lag, d[:1] vs d[:2]
    stage placement around L1, and deferring the s3 copies all just
    RESHUFFLE these gaps (measured 240-245); none net out positive.
    ScalarE busy is pinned ~217us in every variant.
  - Head-to-first-ACT1 varies ~10.5-15us RUN TO RUN (DGE queue startup
    +-2us dominates); w1a enqueue-order tweaks and N_BURST 2 vs 5 are
    within that noise.  Don't chase the head without many repeat runs.
  - Moving L1 before the stage drains starves the PE during ACT1-read
    (PE needs ~2.5us/slot of the ~3.3us slot) -- don't.
Third round (2 more iterations, both reverted): merging the tail singles
into per-tile 2/3-pair groups + a 3-pair (10,11,12) group cut ScalarE
busy 218->211us EXACTLY as predicted, but the coarser ACT2 granularity
deepened the boundary starvation (gaps 15->24us, net 244-248us); adding
an early per-tile group-0 to refill the boundary backlog made it worse
still (248us).  CONCLUSION: ScalarE busy and the boundary gaps trade ~1:1
via ACT2 granularity in this pipeline; ~2048-col ACT2 with per-pair tail
singles is the equilibrium.
Fourth round: TESTED the ps2-triple-buffer route -- 3-pair ps1 fills
(13 = 3+3+3+3+1, quadrants stay distinct so L1 still co-issues, xt
unchanged) free a bank for ps2 bufs=3.  It LOST ~10us cool-equivalent:
the extra ACT1 op/tile plus 5-slot pipeline friction outweigh the added
DVE->L2 slack (hot-run 297us, busy/1.22 ~218.5 + gaps 20 vs eq 217+15).
The 8-bank split 4/2/2 with this emission order is the equilibrium;
no PSUM rebalance improves it.
Fifth round (THE WIN, -3.5us -> 235.5/237.0 verified): the boundary
starvation had an untested PRODUCTION-side lever -- the drain caps
[3,3,3,4] dated from when drains carried ScalarE ACT2s; with drains now
PE+DVE-only, front-loading to [4,4,4,1] completes each group's stages
~1 slot earlier, halving the boundary gaps (2x562ns -> 1x562ns per
super, gapsum 15.1 -> 11.7us).  Remaining: one 562ns gap per super (the
bare ACT1->L1->ACT1 serialization in the one still-empty slot).
Remaining theoretical headroom: ramp ~5us (cold PE, fundamentally HAM),
exit barrier ~6.5us (framework), steady gaps ~7us (above), head ~3us --
but each lever measured neutral-to-negative at this op-granularity.

Older hard-won notes that still hold:
  - Build with bacc.Bacc; walrus allows ONE embedded wait per instruction.
  - tile_position column offsets fail walrus; only row tiling.  Matmul
    lhsT/rhs base partition must be 0/32/64/96; PSUM-dst matmuls fit ONE
    2KB bank (N <= 512 fp32).  K < 32 streams the full 32-row quadrant
    (garbage rows x stale weights), keep K=32 slices zeroed.
  - DVE memset/ops need 32-aligned partition base; no PSUM-source DMA.
  - Dead end: dense [nt, 4, 48, TILE] xt + strided DMA (races / NaN /
    device crash; details in git history of this docstring).
"""

import os
import sys

import numpy as np


def _ensure_path():
    for p in ("/opt/trn_rl_repo",):
        if p not in sys.path:
            sys.path.insert(0, p)


try:
    import concourse.bass as bass  # noqa: F401
except ImportError:
    _ensure_path()

import concourse.bass as bass  # noqa: F401
import concourse.bacc as bacc
import concourse.mybir as mybir
import concourse.tile as tile
from contextlib import ExitStack
from concourse.bass_utils import run_bass_kernel_spmd

NCORES = 8
B = 65536
BC = B // NCORES
T = 26
C = 5
H = 64
NPAIR = 13
TILE = 512
F32 = mybir.dt.float32
BF16 = mybir.dt.bfloat16
FP16 = mybir.dt.float16
TANH = mybir.ActivationFunctionType.Tanh
ADD = mybir.AluOpType.add
MULT = mybir.AluOpType.mult
MIN = mybir.AluOpType.min
MAX = mybir.AluOpType.max

PLANE_PAIRS = [[0, 1, 2, 3], [4, 5, 6, 7], [8, 9, 10, 11], [12]]
DVE_PLANES = ()  # planes whose layer-1 tanh runs on the Vector engine

# Rational tanh fit on [-4.5, 4.5] (max abs err 1.9e-4 incl. saturation):
#   n = ((u+G1)*u+G0)*x ; d = ((u+D1)*u+D0)*KQ ; tanh ~ n/d ; u = clamp(x)^2
TG1, TG0 = 144.13813397, 1387.97534909
TD1, TD0 = 32.89048084, 75.28148013
TKQ = 18.43868257
TCLAMP = 4.5

# Drain counts per gap (after each plane step).  [3,3,3,4] was tuned when
# each drain carried a ScalarE ACT2; drains are now PE+DVE stages only,
# and front-loading [4,4,4,1] completes each ACT2 group's stages ~1 slot
# earlier, shrinking the boundary window where act2q runs dry.
GAPS_SCALAR = [4, 4, 4, 1]
GAPS_DVE1 = [3, 3, 1, 6]  # DVE_PLANES = ()

LAST_RESULTS = None

# HAM warm-keeping: the PE's clock-gate (HAM) drops to K=4/8 (1.2 GHz)
# whenever PE activity in a ~3.4us window looks idle; this kernel's PE duty
# (~50%, ScalarE-bound) makes it oscillate (baseline: 63us of matmuls at
# half clock, 26us cold start).  Cheap zero matmuls (K=32 zeroed SBUF,
# N-col, out = row 0 of a PSUM buffer that the next real matmul overwrites
# with start=True) keep the activity window busy so real matmuls run 2x.
# Mid-kernel HAM dummies are DEAD: an fp32 source makes each dummy a
# 2-pass fp32_mode=LOW_HIGH matmul (~2.1us!), and even bf16 dummies cost
# ~400-600ns of in-order PE queue time (LDW+MM overhead floor, not
# ~110ns).  24+2-per-L2 dummies measured 692us (2.7x WORSE); an 8-dummy
# fp32 startup burst alone added ~10us of head delay.
# A BF16 startup-only burst is different: it fills the otherwise-idle
# 0..9us DMA wait so HAM is warm (K=8/8) when the first L1 runs --
# without it the first ~34us of matmuls measured at half clock.
# Engine instruction streams only arrive at ~6-7.5us (iqueue DMA), so a
# burst can start no earlier than ~7us.  5 dummies cover the w1a DMA wait;
# cutting to 2 did NOT improve the measured head (DGE startup variance
# ±2us dominates).
N_BURST = 5
DUMS_PER_L2 = 0


def build_nc(bc=BC):
    nt = bc // TILE
    nc = bacc.Bacc("TRN2", target_bir_lowering=False, debug=False)
    xt_d = nc.dram_tensor("xt", [nt, 4, 128, TILE], BF16, kind="ExternalInput")
    w1a_d = nc.dram_tensor("w1a", [128, NPAIR * 128], BF16, kind="ExternalInput")
    w2a_d = nc.dram_tensor("w2a", [128, NPAIR * 128], BF16, kind="ExternalInput")
    w3a_d = nc.dram_tensor("w3a", [128, NPAIR * 128], BF16, kind="ExternalInput")
    b2a_d = nc.dram_tensor("b2a", [128, NPAIR], F32, kind="ExternalInput")
    out = nc.dram_tensor("out", [T, bc], F32, kind="ExternalOutput")

    gaps = GAPS_DVE1 if DVE_PLANES == (1,) else GAPS_SCALAR

    with tile.TileContext(nc) as tc, ExitStack() as ctx:
        wpool = ctx.enter_context(tc.tile_pool(name="weights", bufs=1))
        xtpool = ctx.enter_context(tc.tile_pool(name="xt", bufs=4))
        h1pool = ctx.enter_context(tc.tile_pool(name="h1", bufs=6))
        h2pool = ctx.enter_context(tc.tile_pool(name="h2", bufs=3))
        zpool = ctx.enter_context(tc.tile_pool(name="z2", bufs=1))
        s3pool = ctx.enter_context(tc.tile_pool(name="s3", bufs=2))
        dvpool = ctx.enter_context(tc.tile_pool(name="dv", bufs=1))
        pp1 = ctx.enter_context(tc.tile_pool(name="ps1", bufs=1, space="PSUM"))
        pp2 = ctx.enter_context(tc.tile_pool(name="ps2", bufs=2, space="PSUM"))
        pp3 = ctx.enter_context(tc.tile_pool(name="ps3", bufs=2, space="PSUM"))

        # Setup DMAs.  One hardware DGE queue moves only ~77-180GB/s, so
        # the ~1.8MB of setup data is spread over three queues (scalar hw,
        # gpsimd hw, sync sw) ORDERED BY NEED: tile-0 slot q needs
        # xt plane q + w1a cols 512q:512(q+1) at ~10.3+1.3q us; the first
        # L2 drains need only w2a's leading pair blocks.  (A single
        # unsplit DMA also makes every consumer wait for its LAST byte --
        # the first L2 measured a 5us stall on full-w2a before splitting.)
        # The w1a enqueues go FIRST on ScalarE's stream: the ~2.7us tanh
        # table load + warmup otherwise delays the transfers' start.
        w1a = wpool.tile([128, NPAIR * 128], BF16)
        w2a = wpool.tile([128, NPAIR * 128], BF16)
        w3a = wpool.tile([128, NPAIR * 128], BF16)
        b2a = wpool.tile([128, NPAIR], F32)
        for q in range(4):
            lo, hi = 512 * q, min(512 * (q + 1), NPAIR * 128)
            nc.scalar.dma_start(out=w1a[:, lo:hi], in_=w1a_d[:, lo:hi])

        # warm the ACT tanh table while the setup DMAs run
        wrm = wpool.tile([1, 1], F32)
        nc.vector.memset(wrm, 0.0)
        nc.scalar.activation(out=wrm, in_=wrm, func=TANH)

        if N_BURST or DUMS_PER_L2:
            # zeroed source block for HAM warm-keeping dummy matmuls (K=32
            # so the PE streams a clean full quadrant, bf16 to avoid the
            # 2-pass fp32 matmul mode)
            dz = dvpool.tile([32, TILE], BF16, name="dz")
            nc.vector.memset(dz, 0.0)

        # ---- software-pipeline state ----
        fifo = []  # entries: (ready_key, tile_idx, pair, k, h1_handle)
        l3q = []  # (tile, pair, h2_slice) L3 matmuls pending emission
        s3q = []  # (tile, ps3_handle) output copies deferred to slot end
        ps3_state = {}  # tile_idx -> [ps3_handle, n_emitted]
        z2_cur = {}  # group -> z2 tile being staged for the current super-tile
        stage_cnt = {}  # group -> stages emitted this super-tile
        act2q = []  # (last_tile, low_pair, z2, npairs, ntl) awaiting ACT2
        act2_ready = {}  # (last_tile, low_pair) -> slot key when ACT2 may fire

        def emit_dummies(ps, n, ncols):
            # zero matmuls into row 0 of `ps` (overwritten by the next real
            # start=True matmul); deps resolved long ago, so these fill PE
            # idle and hold the HAM activity window busy.
            for _ in range(n):
                nc.tensor.matmul(
                    out=ps[0:1, 0:ncols],
                    lhsT=dz[:, 0:1],
                    rhs=dz[:, 0:ncols],
                    start=True,
                    stop=True,
                )

        def emit_l2(ent):
            # L2 matmul for (pair, tile) + DVE stage of the fp16 pre-act
            # WITH the b2 bias folded in (tensor_scalar_add broadcasts the
            # per-partition bias at the same cost as a copy).  The tanh
            # (ACT2) then needs NO bias port, so one op spans a whole
            # 4-pair PLANE GROUP x 2 tiles (4096 cols) -- 4 ScalarE ops
            # per super-tile instead of 13.
            _, i, p, k, h1 = ent
            ps2 = pp2.tile([128, TILE], F32, tag="ps2")
            nc.tensor.matmul(
                out=ps2,
                lhsT=w2a[:, 128 * p : 128 * (p + 1)],
                rhs=h1[:, k, :],
                start=True,
                stop=True,
            )
            bias = b2a[:, p : p + 1]
            if i >= nt - 2:
                # last 2 tiles: per-pair single-tile ACT2s so the tanh
                # tail starts during tile nt-2 instead of bunching after
                # the final ACT1.  (Merging these into per-tile 2/3-pair
                # groups, and merging pair 12 into a 3-pair group, DID cut
                # ScalarE busy 218->211us as predicted -- but the coarser
                # ACT2 granularity widened the boundary-starvation gaps by
                # more: 244-248us net.  Reverted.)
                z2t = zpool.tile([128, 1, 2, TILE], FP16, tag=f"z2s_{p}")
                nc.vector.tensor_scalar_add(
                    out=z2t[:, 0, 0, :], in0=ps2, scalar1=bias
                )
                act2_ready[(i, p)] = next_key(cur_slot[0])
                act2q.append((i, p, z2t, 1, 1))
            else:
                # 2-pair groups: 4-pair groups saved ~10us more ScalarE
                # busy but the 8-stage gating + 8-MM L3 bursts idle-bunched
                # the PE (throttle 101us, +23us of ScalarE stalls -> 257us
                # total).  2048-col ops are the sweet spot.
                g, kk = divmod(p, 2)
                ngp = 1 if p == NPAIR - 1 else 2
                if g not in z2_cur:
                    z2_cur[g] = zpool.tile(
                        [128, ngp, 2, TILE], FP16, tag=f"z2g_{g}",
                        name=f"z2g_{g}",
                    )
                nc.vector.tensor_scalar_add(
                    out=z2_cur[g][:, kk, i % 2, :], in0=ps2, scalar1=bias
                )
                stage_cnt[g] = stage_cnt.get(g, 0) + 1
                if stage_cnt[g] == 2 * ngp:
                    stage_cnt[g] = 0
                    # 1 slot of lag: with the front-loaded [4,4,4,1]
                    # drains the gating DVE stage completes a slot
                    # earlier, so groups can be consumable sooner --
                    # filling the one remaining act2q-empty boundary slot
                    act2_ready[(i, 2 * g)] = next_key(cur_slot[0])
                    act2q.append((i, 2 * g, z2_cur.pop(g), ngp, 2))

        def emit_act2s(now, budget):
            # pop ready bias-free ACT2 groups (tanh over npairs x ntl
            # tiles) and queue their L3 matmuls
            n = 0
            j = 0
            while j < len(act2q) and n < budget:
                ilast, plow, z2, npairs, ntl = act2q[j]
                if act2_ready.get((ilast, plow), (0, 0)) <= now:
                    act2q.pop(j)
                    n += 1
                    h2 = h2pool.tile([128, 2, 2, TILE], BF16, tag="h2")
                    nc.scalar.activation(
                        out=h2[:, 0:npairs, 0:ntl, :],
                        in_=z2[:, 0:npairs, 0:ntl, :],
                        func=TANH,
                    )
                    # L3s deferred to the next slot's emit_l3s() so the
                    # in-order PE queue never stalls waiting on this ACT2
                    for kk in range(npairs):
                        for t in range(ntl):
                            l3q.append(
                                (ilast - ntl + 1 + t, plow + kk, h2[:, kk, t, :])
                            )
                else:
                    j += 1

        def emit_l3s():
            while l3q:
                i, p, h2t = l3q.pop(0)
                if i not in ps3_state:
                    ps3_state[i] = [
                        pp3.tile([128, TILE], F32, tag="ps3", name="ps3"),
                        0,
                    ]
                st = ps3_state[i]
                st[1] += 1
                nc.tensor.matmul(
                    out=st[0],
                    lhsT=w3a[:, 128 * p : 128 * (p + 1)],
                    rhs=h2t,
                    start=(st[1] == 1),
                    stop=(st[1] == NPAIR),
                )
                if st[1] == NPAIR:
                    # defer the DVE copy + out-DMA to the slot END: the
                    # two per-super copies otherwise sit in the DVE queue
                    # AHEAD of the next slot's stages, whose L2s gate the
                    # ps1 refill (measured ~1-2us ScalarE stall/super)
                    s3q.append((i, st[0]))
                    del ps3_state[i]

        def emit_s3s():
            while s3q:
                i, ps3 = s3q.pop(0)
                s3 = s3pool.tile([T, TILE], F32, tag="s3")
                nc.vector.tensor_copy(out=s3, in_=ps3[0:T, :])
                nc.gpsimd.dma_start(
                    out=out[:, i * TILE : (i + 1) * TILE], in_=s3
                )

        cur_slot = [(0, 0)]

        def next_key(sl):
            i, q = sl
            return (i, q + 1) if q < 3 else (i + 1, 0)

        def pop_ready(n, now):
            got = []
            j = 0
            while j < len(fifo) and len(got) < n:
                if fifo[j][0] <= now:
                    got.append(fifo.pop(j))
                else:
                    j += 1
            return got

        def emit_dve_tanh(ps1, h1, npr):
            xc = dvpool.tile([128, 4, TILE], F32, tag="dv_xc")
            u = dvpool.tile([128, 4, TILE], F32, tag="dv_u")
            a = dvpool.tile([128, 4, TILE], F32, tag="dv_a")
            b = dvpool.tile([128, 4, TILE], F32, tag="dv_b")
            xc, u, a, b = (z[:, 0:npr, :] for z in (xc, u, a, b))
            src = ps1[:, 0:npr, :]
            nc.vector.tensor_scalar(
                out=xc, in0=src, scalar1=-TCLAMP, scalar2=TCLAMP, op0=MAX, op1=MIN
            )
            nc.vector.tensor_mul(u, xc, xc)
            nc.vector.scalar_tensor_tensor(
                out=a, in0=u, scalar=TG1, in1=u, op0=ADD, op1=MULT
            )
            nc.vector.scalar_tensor_tensor(
                out=b, in0=a, scalar=TG0, in1=xc, op0=ADD, op1=MULT
            )
            nc.vector.scalar_tensor_tensor(
                out=a, in0=u, scalar=TD1, in1=u, op0=ADD, op1=MULT
            )
            nc.vector.tensor_scalar(
                out=u, in0=a, scalar1=TD0, scalar2=TKQ, op0=ADD, op1=MULT
            )
            nc.vector.reciprocal(out=a, in_=u)
            nc.vector.tensor_mul(h1[:, 0:npr, :], b, a)

        xt_tiles = {}

        def emit_xt_dma(i, split=False):
            # per-plane plain 2D DMAs (a "q p n -> p q n" rearrange lowers
            # to the slow software DGE); planes split across the gpsimd +
            # sync queues so a tile lands in parallel
            xt = xtpool.tile([128, 4, TILE], BF16, tag="xt")
            if split:
                nc.gpsimd.dma_start(out=xt[:, 0, :], in_=xt_d[i, 0])
                for q in (1, 2, 3):
                    nc.scalar.dma_start(out=xt[:, q, :], in_=xt_d[i, q])
            else:
                for q in (0, 2):
                    nc.gpsimd.dma_start(out=xt[:, q, :], in_=xt_d[i, q])
                for q in (1, 3):
                    nc.sync.dma_start(out=xt[:, q, :], in_=xt_d[i, q])
            xt_tiles[i] = xt

        def emit_l1(i, q):
            prs = PLANE_PAIRS[q]
            ps1 = pp1.tile([128, 4, TILE], F32, tag="ps1")
            xt = xt_tiles[i]
            for k, p in enumerate(prs):
                nc.tensor.matmul(
                    out=ps1[:, k, :],
                    lhsT=w1a[32 * k : 32 * k + 32, 128 * p : 128 * (p + 1)],
                    rhs=xt[32 * k : 32 * k + 32, q, :],
                    start=True,
                    stop=True,
                    tile_position=(32 * k, 0),
                )
            return ps1

        # Slot pipeline over (tile, plane). Per slot: the activation for the
        # ps1 filled in the previous slot, then deferred L3s, two L2+ACT2
        # drains, the NEXT slot's L1 matmuls, and the remaining drains --
        # an order that keeps the in-order PE queue from stalling ScalarE.
        slots = [(i, q) for i in range(nt) for q in range(4)]
        # tile-0 xt planes interleaved with w2a chunks on the gpsimd hw
        # queue (~180GB/s), ordered by first use; w3a on the sync queue
        xt0 = xtpool.tile([128, 4, TILE], BF16, tag="xt")
        nc.gpsimd.dma_start(out=xt0[:, 0, :], in_=xt_d[0, 0])
        nc.gpsimd.dma_start(out=xt0[:, 1, :], in_=xt_d[0, 1])
        nc.gpsimd.dma_start(out=b2a, in_=b2a_d[:, :])
        nc.gpsimd.dma_start(out=w2a[:, 0:256], in_=w2a_d[:, 0:256])
        nc.gpsimd.dma_start(out=xt0[:, 2, :], in_=xt_d[0, 2])
        nc.gpsimd.dma_start(out=xt0[:, 3, :], in_=xt_d[0, 3])
        nc.gpsimd.dma_start(out=w2a[:, 256:768], in_=w2a_d[:, 256:768])
        nc.gpsimd.dma_start(out=w2a[:, 768:], in_=w2a_d[:, 768:])
        xt_tiles[0] = xt0
        nc.sync.dma_start(out=w3a, in_=w3a_d[:, :])
        if N_BURST:
            # startup HAM warm-up burst during the w1a/xt DMA wait
            ps_warm = pp2.tile([128, TILE], F32, tag="ps2")
            emit_dummies(ps_warm, N_BURST, TILE)
        ps1 = emit_l1(0, 0)
        for j, (i, q) in enumerate(slots):
            cur_slot[0] = (i, q)
            prs = PLANE_PAIRS[q]
            npr = len(prs)
            h1 = h1pool.tile([128, 4, TILE], BF16, tag="h1")
            if q in DVE_PLANES:
                emit_dve_tanh(ps1, h1, npr)
                ready = (i + 1, 0)  # drainable from next tile's slot 0
            elif i == 0 and npr == 4:
                # ramp: split tile-0's ACT1 into 2-pair halves so the next
                # L1 refill of banks 0-1 overlaps the second half (the PE
                # is cold here and ACT2 backlog hasn't built up yet)
                nc.scalar.activation(
                    out=h1[:, 0:2, :], in_=ps1[:, 0:2, :], func=TANH
                )
                nc.scalar.activation(
                    out=h1[:, 2:4, :], in_=ps1[:, 2:4, :], func=TANH
                )
                ready = (i, q + 1) if q < 3 else (i + 1, 0)
            else:
                nc.scalar.activation(
                    out=h1[:, 0:npr, :], in_=ps1[:, 0:npr, :], func=TANH
                )
                # plane q drains from the gap after plane q+1 (wrapping);
                # on the last tile drain immediately to shorten the tail
                if i == nt - 1:
                    ready = (i, q)
                else:
                    ready = (i, q + 1) if q < 3 else (i + 1, 0)
            for k, p in enumerate(prs):
                fifo.append((ready, i, p, k, h1))
            emit_act2s((i, q), 1)
            emit_l3s()
            d = pop_ready(gaps[q] if i < nt - 1 else 6, (i, q))
            for ent in d[:2]:
                emit_l2(ent)
            if q == 1 and i + 1 < nt:
                emit_xt_dma(i + 1)
            if j + 1 < len(slots):
                ps1 = emit_l1(*slots[j + 1])
            for ent in d[2:]:
                emit_l2(ent)
            emit_s3s()
            emit_act2s((i, q), 1)

        # flush the pipeline tail
        flush_now = (nt + 9, 9)
        while fifo or act2q:
            emit_l3s()
            emit_s3s()
            for ent in pop_ready(3, flush_now):
                emit_l2(ent)
            emit_act2s(flush_now, 3)
        emit_l3s()
        emit_s3s()
    return nc


def pack_weights(W1, b1, W2, b2, W3, b3):
    W1 = np.asarray(W1, dtype=np.float32)
    b1 = np.asarray(b1, dtype=np.float32)
    W2 = np.asarray(W2, dtype=np.float32)
    b2 = np.asarray(b2, dtype=np.float32)
    W3 = np.asarray(W3, dtype=np.float32)
    import ml_dtypes

    bf16 = ml_dtypes.bfloat16
    w1a = np.zeros((128, NPAIR * 128), np.float32)
    w2a = np.zeros((128, NPAIR * 128), np.float32)
    w3a = np.zeros((128, NPAIR * 128), np.float32)
    b2a = np.zeros((128, NPAIR), np.float32)
    for t in range(T):
        p, e = divmod(t, 2)
        k = p % 4
        w1a[32 * k + 5 * e : 32 * k + 5 * e + 5,
            128 * p + 64 * e : 128 * p + 64 * e + 64] = W1[t]
        w1a[32 * k + 10, 128 * p + 64 * e : 128 * p + 64 * e + 64] = b1[t]
        w2a[64 * e : 64 * e + 64, 128 * p + 64 * e : 128 * p + 64 * e + 64] = W2[t]
        # L3: pair p writes ps3 rows 2p, 2p+1 -> nonzero cols 2p+e in-block
        w3a[64 * e : 64 * e + 64, 128 * p + 2 * p + e] = W3[t][:, 0]
        b2a[64 * e : 64 * e + 64, p] = b2[t]
    return {
        "w1a": w1a.astype(bf16),
        "w2a": w2a.astype(bf16),
        "w3a": w3a.astype(bf16),
        "b2a": b2a,
    }


def pack_xt(features_core):
    """[bc, 26, 5] -> [nt, 4, 128, TILE] pair-aligned transposed layout."""
    bc = features_core.shape[0]
    nt = bc // TILE
    ff = np.asarray(features_core, np.float32).reshape(nt, TILE, T, C)
    import ml_dtypes

    xt = np.zeros((nt, 4, 128, TILE), ml_dtypes.bfloat16)
    for t in range(T):
        p, e = divmod(t, 2)
        q, k = divmod(p, 4)
        xt[:, q, 32 * k + 5 * e : 32 * k + 5 * e + 5, :] = ff[:, :, t, :].swapaxes(
            1, 2
        )
    for p in range(NPAIR):
        q, k = divmod(p, 4)
        xt[:, q, 32 * k + 2 * C, :] = 1.0
    return xt


def kernel(features, W1, b1, W2, b2, W3, b3):
    global LAST_RESULTS
    features = np.asarray(features, dtype=np.float32)
    b3 = np.asarray(b3, dtype=np.float32)
    ins = pack_weights(W1, b1, W2, b2, W3, b3)
    nc = build_nc(BC)
    nc.finalize()
    in_maps = []
    for c in range(NCORES):
        m = dict(ins)
        m["xt"] = pack_xt(features[c * BC : (c + 1) * BC])
        in_maps.append(m)
    trace = bool(int(os.environ.get("KERNEL_TRACE", "0")))
    # The first execution of a freshly loaded NEFF intermittently faults with
    # NRT_EXEC_UNIT_UNRECOVERABLE; a retry on the recovered device succeeds.
    last_exc = None
    for attempt in range(3):
        try:
            res = run_bass_kernel_spmd(
                nc, in_maps, list(range(NCORES)), trace=trace
            )
            LAST_RESULTS = res
            # out is type-major [26, bc]; transpose + b3 on the host
            return np.concatenate(
                [
                    res.results[c]["out"].T + b3[:, 0][None, :]
                    for c in range(NCORES)
                ],
                axis=0,
            )
        except Exception as e:  # noqa: BLE001
            last_exc = e
            import time as _time

            _time.sleep(5.0 * (attempt + 1))
    raise last_exc



# revision 75
# speedup vs baseline: 1.0117x; 1.0117x over previous
"""Trainium2 Bass kernel for nn_AiMAiPartiallyConnectedLayers.

26 independent MLPs (5 -> 64 -> 64 -> 1, tanh) applied per node type over a
batch of 65536 samples; output [B, 26] fp32.  Pure data parallel over 8
NeuronCores (8192 samples each); ~235.5-237 us HW time (cool device;
baseline was 257.8), rel err ~3.3e-3 (bf16 matmul precision).

Design (ScalarE/tanh is the bottleneck engine; its ~215us busy IS the
kernel -- everything else hides behind it):
  - Types in 13 pairs, packed block-diagonal [128, 128] bf16 per pair.
    Input pre-transposed host-side to xt[tile, q, 128, 512]; plane q holds
    pairs 4q+k on partition rows 32k:32k+11 (10 channel rows + a ones row
    folding b1).  Layer-1 matmuls: K=32 with row tile_position (32k, 0);
    the 4 matmuls of a plane co-issue on disjoint PE row-quadrants.
  - ACT1 is one op per PLANE over a 4-bank PSUM region [128, 4, 512]
    (2048 cols).  ps1 is a single 4-bank buffer; refills hide under the
    ACT2 drains between ACT1 ops.
  - L2 path: per (pair, tile) matmul into ps2 [128,512], then a DVE
    tensor_scalar_add stages the pre-activation to SBUF **fp16** WITH the
    b2 bias folded in (per-partition AP scalar broadcasts at copy cost).
    ACT2 is then a BIAS-FREE tanh from SBUF, so one op spans a 2-pair
    group x 2-tile super-tile (2048 cols) -- ~6.2us/tile of ACT2 vs 7.8
    for per-pair-per-tile PSUM ops.  Last 2 tiles use per-pair single-
    tile ACT2s so the tanh tail starts early (avoids +4us tail bunching).
  - Layer-3 weights for pair p have nonzero columns 2p, 2p+1 inside the
    pair block, so ALL 13 pairs accumulate into rows 0:26 of ONE PSUM bank
    per tile (start/stop flags).  One DVE copy moves [26, 512] to SBUF,
    one DMA writes a type-major DRAM output [26, bc]; the HOST transposes
    and adds b3.  L3s are deferred one slot after their ACT2 so the
    in-order PE queue never stalls on ScalarE.
  - Slot pipeline over (tile, plane): ACT1(q) | 1 ACT2 pop | L3s(lag-1) |
    2 stage drains | L1(next plane) | remaining drains | 1 ACT2 pop.
    Stage-drain FIFO keys as before; ACT2 groups become ready 2 slots
    after their last stage (DVE slack).
  - PSUM: ps1 4 banks (single) + ps2 2x1 + ps3 2x1 = 8 banks exactly.
  - Setup DMAs ordered by need across 3 queues (see comment in code);
    tile-0 ACT1s split in 2-pair halves to soften the cold-PE ramp.

Measured cost models (this session, from ntff profiles -- the v1
docstring's ACT model was WRONG):
  - ACTIVATE: dur ~= 0.846ns/col + 167ns fixed (512col=636, 1024=1114,
    2048=1970, 4096=3630).  Marginal dominates; ScalarE floor for all
    tanh cols is ~180us/core.  Op count only matters ~167ns at a time.
  - DVE COPY/TENSOR_SCALAR [128,512] fp32->fp16: ~690ns (~0.84ns/col).
  - bf16 MM N=512 ~213ns warm, ~600 cold; LDW 128x128 ~100ns.  fp32
    matmuls lower to TWO passes (fp32_mode=LOW_HIGH, ~1us each) -- never
    matmul fp32 casually.
  - HAM: PE clock-gate DEFAULTS to 4/8 (1.2GHz); only ~3.4us of sustained
    PE busy releases it, any ~3.4us idle window re-throttles.  This
    ScalarE-bound kernel has PE duty ~60-70% -> oscillates (~28-63us of
    matmuls run at half clock).  Dummy-matmul warming FAILS: each dummy
    costs 400-600ns of in-order PE queue (692us disaster at 860 dummies);
    engines can't even start before their iqueue DMA lands (~6-7.5us).
  - DMA: one hw DGE queue ~77-180GB/s; sync(SP) queue is a SOFTWARE DGE
    (slower).  dma_start allowed only on gpsimd/sync/scalar.  Each
    enqueue instruction costs ~600ns on the issuing engine; a consumer
    waits for the LAST byte of the whole DMA it depends on -> split
    setup weights by first-use and order by need.
  - Thermal: 4+ back-to-back HW runs downclock EVERYTHING ~20-25%
    (285us vs 239 for the identical NEFF).  Sleep >=2min before judging.

Tried and rejected this session:
  - HAM dummy matmuls (mid-kernel or long startup bursts): see above.
  - 4-pair ACT2 groups (4096 cols): ScalarE busy 206.7us (best seen!) but
    8-stage gating + 8-MM L3 bursts idle-bunch the PE -> throttle 101us,
    +23us ScalarE stalls -> 257us net.  2-pair groups are the sweet spot.
  - DVE rational tanh (reciprocal ~6ns/col) and polynomial tanh (~8 ops
    x ~0.85ns/col + 260ns/op fixed ~= 5-9ns/col effective): both lose;
    ScalarE does tanh at 0.85ns/col.
  - PSUM-only ACT2 merges (2-tile [128,2,512] ps2): blocked by the 8-bank
    wall (ps1 4 + ps2 2x2 + ps3 2 = 10) unless ACT1 drops to 2-pair
    groups, which costs more than the merge saves under the REAL ACT
    model (fixed is 167ns, not 420).
Second-round findings (3 more HW iterations, all reverted to this
config which measured 238.9/239.1/239.2 across independent cool runs):
  - Steady-state ScalarE gaps are ~565ns x2 per super-tile boundary
    (~7us total): when act2q runs dry there, ACT1 waits on the ps1
    refill chain L1 <- L2-drains <- DVE-ps2-frees.  Pop-budget leveling
    (1/slot + conditional), 1-slot vs 2-slot ACT2 lag, d[:1] vs d[:2]
    stage placement around L1, and deferring the s3 copies all just
    RESHUFFLE these gaps (measured 240-245); none net out positive.
    ScalarE busy is pinned ~217us in every variant.
  - Head-to-first-ACT1 varies ~10.5-15us RUN TO RUN (DGE queue startup
    +-2us dominates); w1a enqueue-order tweaks and N_BURST 2 vs 5 are
    within that noise.  Don't chase the head without many repeat runs.
  - Moving L1 before the stage drains starves the PE during ACT1-read
    (PE needs ~2.5us/slot of the ~3.3us slot) -- don't.
Third round (2 more iterations, both reverted): merging the tail singles
into per-tile 2/3-pair groups + a 3-pair (10,11,12) group cut ScalarE
busy 218->211us EXACTLY as predicted, but the coarser ACT2 granularity
deepened the boundary starvation (gaps 15->24us, net 244-248us); adding
an early per-tile group-0 to refill the boundary backlog made it worse
still (248us).  CONCLUSION: ScalarE busy and the boundary gaps trade ~1:1
via ACT2 granularity in this pipeline; ~2048-col ACT2 with per-pair tail
singles is the equilibrium.
Fourth round: TESTED the ps2-triple-buffer route -- 3-pair ps1 fills
(13 = 3+3+3+3+1, quadrants stay distinct so L1 still co-issues, xt
unchanged) free a bank for ps2 bufs=3.  It LOST ~10us cool-equivalent:
the extra ACT1 op/tile plus 5-slot pipeline friction outweigh the added
DVE->L2 slack (hot-run 297us, busy/1.22 ~218.5 + gaps 20 vs eq 217+15).
The 8-bank split 4/2/2 with this emission order is the equilibrium;
no PSUM rebalance improves it.
Fifth round (THE WIN, -3.5us -> 235.5/237.0 verified): the boundary
starvation had an untested PRODUCTION-side lever -- the drain caps
[3,3,3,4] dated from when drains carried ScalarE ACT2s; with drains now
PE+DVE-only, front-loading to [4,4,4,1] completes each group's stages
~1 slot earlier, halving the boundary gaps (2x562ns -> 1x562ns per
super, gapsum 15.1 -> 11.7us).  Remaining: one 562ns gap per super (the
bare ACT1->L1->ACT1 serialization in the one still-empty slot).
Dropping the ACT2 lag to 1 slot ON TOP of [4,4,4,1] to fill that last
slot measured 239.3 (worse) -- the 2-slot lag is load-bearing even with
front-loaded drains.  [4,4,4,1] + 2-slot lag is the final equilibrium.
Remaining theoretical headroom: ramp ~5us (cold PE, fundamentally HAM),
exit barrier ~6.5us (framework), steady gaps ~7us (above), head ~3us --
but each lever measured neutral-to-negative at this op-granularity.

Older hard-won notes that still hold:
  - Build with bacc.Bacc; walrus allows ONE embedded wait per instruction.
  - tile_position column offsets fail walrus; only row tiling.  Matmul
    lhsT/rhs base partition must be 0/32/64/96; PSUM-dst matmuls fit ONE
    2KB bank (N <= 512 fp32).  K < 32 streams the full 32-row quadrant
    (garbage rows x stale weights), keep K=32 slices zeroed.
  - DVE memset/ops need 32-aligned partition base; no PSUM-source DMA.
  - Dead end: dense [nt, 4, 48, TILE] xt + strided DMA (races / NaN /
    device crash; details in git history of this docstring).
"""

import os
import sys

import numpy as np


def _ensure_path():
    for p in ("/opt/trn_rl_repo",):
        if p not in sys.path:
            sys.path.insert(0, p)


try:
    import concourse.bass as bass  # noqa: F401
except ImportError:
    _ensure_path()

import concourse.bass as bass  # noqa: F401
import concourse.bacc as bacc
import concourse.mybir as mybir
import concourse.tile as tile
from contextlib import ExitStack
from concourse.bass_utils import run_bass_kernel_spmd

NCORES = 8
B = 65536
BC = B // NCORES
T = 26
C = 5
H = 64
NPAIR = 13
TILE = 512
F32 = mybir.dt.float32
BF16 = mybir.dt.bfloat16
FP16 = mybir.dt.float16
TANH = mybir.ActivationFunctionType.Tanh
ADD = mybir.AluOpType.add
MULT = mybir.AluOpType.mult
MIN = mybir.AluOpType.min
MAX = mybir.AluOpType.max

PLANE_PAIRS = [[0, 1, 2, 3], [4, 5, 6, 7], [8, 9, 10, 11], [12]]
DVE_PLANES = ()  # planes whose layer-1 tanh runs on the Vector engine

# Rational tanh fit on [-4.5, 4.5] (max abs err 1.9e-4 incl. saturation):
#   n = ((u+G1)*u+G0)*x ; d = ((u+D1)*u+D0)*KQ ; tanh ~ n/d ; u = clamp(x)^2
TG1, TG0 = 144.13813397, 1387.97534909
TD1, TD0 = 32.89048084, 75.28148013
TKQ = 18.43868257
TCLAMP = 4.5

# Drain counts per gap (after each plane step).  [3,3,3,4] was tuned when
# each drain carried a ScalarE ACT2; drains are now PE+DVE stages only,
# and front-loading [4,4,4,1] completes each ACT2 group's stages ~1 slot
# earlier, shrinking the boundary window where act2q runs dry.
GAPS_SCALAR = [4, 4, 4, 1]
GAPS_DVE1 = [3, 3, 1, 6]  # DVE_PLANES = ()

LAST_RESULTS = None

# HAM warm-keeping: the PE's clock-gate (HAM) drops to K=4/8 (1.2 GHz)
# whenever PE activity in a ~3.4us window looks idle; this kernel's PE duty
# (~50%, ScalarE-bound) makes it oscillate (baseline: 63us of matmuls at
# half clock, 26us cold start).  Cheap zero matmuls (K=32 zeroed SBUF,
# N-col, out = row 0 of a PSUM buffer that the next real matmul overwrites
# with start=True) keep the activity window busy so real matmuls run 2x.
# Mid-kernel HAM dummies are DEAD: an fp32 source makes each dummy a
# 2-pass fp32_mode=LOW_HIGH matmul (~2.1us!), and even bf16 dummies cost
# ~400-600ns of in-order PE queue time (LDW+MM overhead floor, not
# ~110ns).  24+2-per-L2 dummies measured 692us (2.7x WORSE); an 8-dummy
# fp32 startup burst alone added ~10us of head delay.
# A BF16 startup-only burst is different: it fills the otherwise-idle
# 0..9us DMA wait so HAM is warm (K=8/8) when the first L1 runs --
# without it the first ~34us of matmuls measured at half clock.
# Engine instruction streams only arrive at ~6-7.5us (iqueue DMA), so a
# burst can start no earlier than ~7us.  5 dummies cover the w1a DMA wait;
# cutting to 2 did NOT improve the measured head (DGE startup variance
# ±2us dominates).
N_BURST = 5
DUMS_PER_L2 = 0


def build_nc(bc=BC):
    nt = bc // TILE
    nc = bacc.Bacc("TRN2", target_bir_lowering=False, debug=False)
    xt_d = nc.dram_tensor("xt", [nt, 4, 128, TILE], BF16, kind="ExternalInput")
    w1a_d = nc.dram_tensor("w1a", [128, NPAIR * 128], BF16, kind="ExternalInput")
    w2a_d = nc.dram_tensor("w2a", [128, NPAIR * 128], BF16, kind="ExternalInput")
    w3a_d = nc.dram_tensor("w3a", [128, NPAIR * 128], BF16, kind="ExternalInput")
    b2a_d = nc.dram_tensor("b2a", [128, NPAIR], F32, kind="ExternalInput")
    out = nc.dram_tensor("out", [T, bc], F32, kind="ExternalOutput")

    gaps = GAPS_DVE1 if DVE_PLANES == (1,) else GAPS_SCALAR

    with tile.TileContext(nc) as tc, ExitStack() as ctx:
        wpool = ctx.enter_context(tc.tile_pool(name="weights", bufs=1))
        xtpool = ctx.enter_context(tc.tile_pool(name="xt", bufs=4))
        h1pool = ctx.enter_context(tc.tile_pool(name="h1", bufs=6))
        h2pool = ctx.enter_context(tc.tile_pool(name="h2", bufs=3))
        zpool = ctx.enter_context(tc.tile_pool(name="z2", bufs=1))
        s3pool = ctx.enter_context(tc.tile_pool(name="s3", bufs=2))
        dvpool = ctx.enter_context(tc.tile_pool(name="dv", bufs=1))
        pp1 = ctx.enter_context(tc.tile_pool(name="ps1", bufs=1, space="PSUM"))
        pp2 = ctx.enter_context(tc.tile_pool(name="ps2", bufs=2, space="PSUM"))
        pp3 = ctx.enter_context(tc.tile_pool(name="ps3", bufs=2, space="PSUM"))

        # Setup DMAs.  One hardware DGE queue moves only ~77-180GB/s, so
        # the ~1.8MB of setup data is spread over three queues (scalar hw,
        # gpsimd hw, sync sw) ORDERED BY NEED: tile-0 slot q needs
        # xt plane q + w1a cols 512q:512(q+1) at ~10.3+1.3q us; the first
        # L2 drains need only w2a's leading pair blocks.  (A single
        # unsplit DMA also makes every consumer wait for its LAST byte --
        # the first L2 measured a 5us stall on full-w2a before splitting.)
        # The w1a enqueues go FIRST on ScalarE's stream: the ~2.7us tanh
        # table load + warmup otherwise delays the transfers' start.
        w1a = wpool.tile([128, NPAIR * 128], BF16)
        w2a = wpool.tile([128, NPAIR * 128], BF16)
        w3a = wpool.tile([128, NPAIR * 128], BF16)
        b2a = wpool.tile([128, NPAIR], F32)
        for q in range(4):
            lo, hi = 512 * q, min(512 * (q + 1), NPAIR * 128)
            nc.scalar.dma_start(out=w1a[:, lo:hi], in_=w1a_d[:, lo:hi])

        # warm the ACT tanh table while the setup DMAs run
        wrm = wpool.tile([1, 1], F32)
        nc.vector.memset(wrm, 0.0)
        nc.scalar.activation(out=wrm, in_=wrm, func=TANH)

        if N_BURST or DUMS_PER_L2:
            # zeroed source block for HAM warm-keeping dummy matmuls (K=32
            # so the PE streams a clean full quadrant, bf16 to avoid the
            # 2-pass fp32 matmul mode)
            dz = dvpool.tile([32, TILE], BF16, name="dz")
            nc.vector.memset(dz, 0.0)

        # ---- software-pipeline state ----
        fifo = []  # entries: (ready_key, tile_idx, pair, k, h1_handle)
        l3q = []  # (tile, pair, h2_slice) L3 matmuls pending emission
        s3q = []  # (tile, ps3_handle) output copies deferred to slot end
        ps3_state = {}  # tile_idx -> [ps3_handle, n_emitted]
        z2_cur = {}  # group -> z2 tile being staged for the current super-tile
        stage_cnt = {}  # group -> stages emitted this super-tile
        act2q = []  # (last_tile, low_pair, z2, npairs, ntl) awaiting ACT2
        act2_ready = {}  # (last_tile, low_pair) -> slot key when ACT2 may fire

        def emit_dummies(ps, n, ncols):
            # zero matmuls into row 0 of `ps` (overwritten by the next real
            # start=True matmul); deps resolved long ago, so these fill PE
            # idle and hold the HAM activity window busy.
            for _ in range(n):
                nc.tensor.matmul(
                    out=ps[0:1, 0:ncols],
                    lhsT=dz[:, 0:1],
                    rhs=dz[:, 0:ncols],
                    start=True,
                    stop=True,
                )

        def emit_l2(ent):
            # L2 matmul for (pair, tile) + DVE stage of the fp16 pre-act
            # WITH the b2 bias folded in (tensor_scalar_add broadcasts the
            # per-partition bias at the same cost as a copy).  The tanh
            # (ACT2) then needs NO bias port, so one op spans a whole
            # 4-pair PLANE GROUP x 2 tiles (4096 cols) -- 4 ScalarE ops
            # per super-tile instead of 13.
            _, i, p, k, h1 = ent
            ps2 = pp2.tile([128, TILE], F32, tag="ps2")
            nc.tensor.matmul(
                out=ps2,
                lhsT=w2a[:, 128 * p : 128 * (p + 1)],
                rhs=h1[:, k, :],
                start=True,
                stop=True,
            )
            bias = b2a[:, p : p + 1]
            if i >= nt - 2:
                # last 2 tiles: per-pair single-tile ACT2s so the tanh
                # tail starts during tile nt-2 instead of bunching after
                # the final ACT1.  (Merging these into per-tile 2/3-pair
                # groups, and merging pair 12 into a 3-pair group, DID cut
                # ScalarE busy 218->211us as predicted -- but the coarser
                # ACT2 granularity widened the boundary-starvation gaps by
                # more: 244-248us net.  Reverted.)
                z2t = zpool.tile([128, 1, 2, TILE], FP16, tag=f"z2s_{p}")
                nc.vector.tensor_scalar_add(
                    out=z2t[:, 0, 0, :], in0=ps2, scalar1=bias
                )
                act2_ready[(i, p)] = next_key(cur_slot[0])
                act2q.append((i, p, z2t, 1, 1))
            else:
                # 2-pair groups: 4-pair groups saved ~10us more ScalarE
                # busy but the 8-stage gating + 8-MM L3 bursts idle-bunched
                # the PE (throttle 101us, +23us of ScalarE stalls -> 257us
                # total).  2048-col ops are the sweet spot.
                g, kk = divmod(p, 2)
                ngp = 1 if p == NPAIR - 1 else 2
                if g not in z2_cur:
                    z2_cur[g] = zpool.tile(
                        [128, ngp, 2, TILE], FP16, tag=f"z2g_{g}",
                        name=f"z2g_{g}",
                    )
                nc.vector.tensor_scalar_add(
                    out=z2_cur[g][:, kk, i % 2, :], in0=ps2, scalar1=bias
                )
                stage_cnt[g] = stage_cnt.get(g, 0) + 1
                if stage_cnt[g] == 2 * ngp:
                    stage_cnt[g] = 0
                    # 2 slots of lag so the gating DVE stage never stalls
                    # the ScalarE op
                    act2_ready[(i, 2 * g)] = next_key(next_key(cur_slot[0]))
                    act2q.append((i, 2 * g, z2_cur.pop(g), ngp, 2))

        def emit_act2s(now, budget):
            # pop ready bias-free ACT2 groups (tanh over npairs x ntl
            # tiles) and queue their L3 matmuls
            n = 0
            j = 0
            while j < len(act2q) and n < budget:
                ilast, plow, z2, npairs, ntl = act2q[j]
                if act2_ready.get((ilast, plow), (0, 0)) <= now:
                    act2q.pop(j)
                    n += 1
                    h2 = h2pool.tile([128, 2, 2, TILE], BF16, tag="h2")
                    nc.scalar.activation(
                        out=h2[:, 0:npairs, 0:ntl, :],
                        in_=z2[:, 0:npairs, 0:ntl, :],
                        func=TANH,
                    )
                    # L3s deferred to the next slot's emit_l3s() so the
                    # in-order PE queue never stalls waiting on this ACT2
                    for kk in range(npairs):
                        for t in range(ntl):
                            l3q.append(
                                (ilast - ntl + 1 + t, plow + kk, h2[:, kk, t, :])
                            )
                else:
                    j += 1

        def emit_l3s():
            while l3q:
                i, p, h2t = l3q.pop(0)
                if i not in ps3_state:
                    ps3_state[i] = [
                        pp3.tile([128, TILE], F32, tag="ps3", name="ps3"),
                        0,
                    ]
                st = ps3_state[i]
                st[1] += 1
                nc.tensor.matmul(
                    out=st[0],
                    lhsT=w3a[:, 128 * p : 128 * (p + 1)],
                    rhs=h2t,
                    start=(st[1] == 1),
                    stop=(st[1] == NPAIR),
                )
                if st[1] == NPAIR:
                    # defer the DVE copy + out-DMA to the slot END: the
                    # two per-super copies otherwise sit in the DVE queue
                    # AHEAD of the next slot's stages, whose L2s gate the
                    # ps1 refill (measured ~1-2us ScalarE stall/super)
                    s3q.append((i, st[0]))
                    del ps3_state[i]

        def emit_s3s():
            while s3q:
                i, ps3 = s3q.pop(0)
                s3 = s3pool.tile([T, TILE], F32, tag="s3")
                nc.vector.tensor_copy(out=s3, in_=ps3[0:T, :])
                nc.gpsimd.dma_start(
                    out=out[:, i * TILE : (i + 1) * TILE], in_=s3
                )

        cur_slot = [(0, 0)]

        def next_key(sl):
            i, q = sl
            return (i, q + 1) if q < 3 else (i + 1, 0)

        def pop_ready(n, now):
            got = []
            j = 0
            while j < len(fifo) and len(got) < n:
                if fifo[j][0] <= now:
                    got.append(fifo.pop(j))
                else:
                    j += 1
            return got

        def emit_dve_tanh(ps1, h1, npr):
            xc = dvpool.tile([128, 4, TILE], F32, tag="dv_xc")
            u = dvpool.tile([128, 4, TILE], F32, tag="dv_u")
            a = dvpool.tile([128, 4, TILE], F32, tag="dv_a")
            b = dvpool.tile([128, 4, TILE], F32, tag="dv_b")
            xc, u, a, b = (z[:, 0:npr, :] for z in (xc, u, a, b))
            src = ps1[:, 0:npr, :]
            nc.vector.tensor_scalar(
                out=xc, in0=src, scalar1=-TCLAMP, scalar2=TCLAMP, op0=MAX, op1=MIN
            )
            nc.vector.tensor_mul(u, xc, xc)
            nc.vector.scalar_tensor_tensor(
                out=a, in0=u, scalar=TG1, in1=u, op0=ADD, op1=MULT
            )
            nc.vector.scalar_tensor_tensor(
                out=b, in0=a, scalar=TG0, in1=xc, op0=ADD, op1=MULT
            )
            nc.vector.scalar_tensor_tensor(
                out=a, in0=u, scalar=TD1, in1=u, op0=ADD, op1=MULT
            )
            nc.vector.tensor_scalar(
                out=u, in0=a, scalar1=TD0, scalar2=TKQ, op0=ADD, op1=MULT
            )
            nc.vector.reciprocal(out=a, in_=u)
            nc.vector.tensor_mul(h1[:, 0:npr, :], b, a)

        xt_tiles = {}

        def emit_xt_dma(i, split=False):
            # per-plane plain 2D DMAs (a "q p n -> p q n" rearrange lowers
            # to the slow software DGE); planes split across the gpsimd +
            # sync queues so a tile lands in parallel
            xt = xtpool.tile([128, 4, TILE], BF16, tag="xt")
            if split:
                nc.gpsimd.dma_start(out=xt[:, 0, :], in_=xt_d[i, 0])
                for q in (1, 2, 3):
                    nc.scalar.dma_start(out=xt[:, q, :], in_=xt_d[i, q])
            else:
                for q in (0, 2):
                    nc.gpsimd.dma_start(out=xt[:, q, :], in_=xt_d[i, q])
                for q in (1, 3):
                    nc.sync.dma_start(out=xt[:, q, :], in_=xt_d[i, q])
            xt_tiles[i] = xt

        def emit_l1(i, q):
            prs = PLANE_PAIRS[q]
            ps1 = pp1.tile([128, 4, TILE], F32, tag="ps1")
            xt = xt_tiles[i]
            for k, p in enumerate(prs):
                nc.tensor.matmul(
                    out=ps1[:, k, :],
                    lhsT=w1a[32 * k : 32 * k + 32, 128 * p : 128 * (p + 1)],
                    rhs=xt[32 * k : 32 * k + 32, q, :],
                    start=True,
                    stop=True,
                    tile_position=(32 * k, 0),
                )
            return ps1

        # Slot pipeline over (tile, plane). Per slot: the activation for the
        # ps1 filled in the previous slot, then deferred L3s, two L2+ACT2
        # drains, the NEXT slot's L1 matmuls, and the remaining drains --
        # an order that keeps the in-order PE queue from stalling ScalarE.
        slots = [(i, q) for i in range(nt) for q in range(4)]
        # tile-0 xt planes interleaved with w2a chunks on the gpsimd hw
        # queue (~180GB/s), ordered by first use; w3a on the sync queue
        xt0 = xtpool.tile([128, 4, TILE], BF16, tag="xt")
        nc.gpsimd.dma_start(out=xt0[:, 0, :], in_=xt_d[0, 0])
        nc.gpsimd.dma_start(out=xt0[:, 1, :], in_=xt_d[0, 1])
        nc.gpsimd.dma_start(out=b2a, in_=b2a_d[:, :])
        nc.gpsimd.dma_start(out=w2a[:, 0:256], in_=w2a_d[:, 0:256])
        nc.gpsimd.dma_start(out=xt0[:, 2, :], in_=xt_d[0, 2])
        nc.gpsimd.dma_start(out=xt0[:, 3, :], in_=xt_d[0, 3])
        nc.gpsimd.dma_start(out=w2a[:, 256:768], in_=w2a_d[:, 256:768])
        nc.gpsimd.dma_start(out=w2a[:, 768:], in_=w2a_d[:, 768:])
        xt_tiles[0] = xt0
        nc.sync.dma_start(out=w3a, in_=w3a_d[:, :])
        if N_BURST:
            # startup HAM warm-up burst during the w1a/xt DMA wait
            ps_warm = pp2.tile([128, TILE], F32, tag="ps2")
            emit_dummies(ps_warm, N_BURST, TILE)
        ps1 = emit_l1(0, 0)
        for j, (i, q) in enumerate(slots):
            cur_slot[0] = (i, q)
            prs = PLANE_PAIRS[q]
            npr = len(prs)
            h1 = h1pool.tile([128, 4, TILE], BF16, tag="h1")
            if q in DVE_PLANES:
                emit_dve_tanh(ps1, h1, npr)
                ready = (i + 1, 0)  # drainable from next tile's slot 0
            elif i == 0 and npr == 4:
                # ramp: split tile-0's ACT1 into 2-pair halves so the next
                # L1 refill of banks 0-1 overlaps the second half (the PE
                # is cold here and ACT2 backlog hasn't built up yet)
                nc.scalar.activation(
                    out=h1[:, 0:2, :], in_=ps1[:, 0:2, :], func=TANH
                )
                nc.scalar.activation(
                    out=h1[:, 2:4, :], in_=ps1[:, 2:4, :], func=TANH
                )
                ready = (i, q + 1) if q < 3 else (i + 1, 0)
            else:
                nc.scalar.activation(
                    out=h1[:, 0:npr, :], in_=ps1[:, 0:npr, :], func=TANH
                )
                # plane q drains from the gap after plane q+1 (wrapping);
                # on the last tile drain immediately to shorten the tail
                if i == nt - 1:
                    ready = (i, q)
                else:
                    ready = (i, q + 1) if q < 3 else (i + 1, 0)
            for k, p in enumerate(prs):
                fifo.append((ready, i, p, k, h1))
            emit_act2s((i, q), 1)
            emit_l3s()
            d = pop_ready(gaps[q] if i < nt - 1 else 6, (i, q))
            for ent in d[:2]:
                emit_l2(ent)
            if q == 1 and i + 1 < nt:
                emit_xt_dma(i + 1)
            if j + 1 < len(slots):
                ps1 = emit_l1(*slots[j + 1])
            for ent in d[2:]:
                emit_l2(ent)
            emit_s3s()
            emit_act2s((i, q), 1)

        # flush the pipeline tail
        flush_now = (nt + 9, 9)
        while fifo or act2q:
            emit_l3s()
            emit_s3s()
            for ent in pop_ready(3, flush_now):
                emit_l2(ent)
            emit_act2s(flush_now, 3)
        emit_l3s()
        emit_s3s()
    return nc


def pack_weights(W1, b1, W2, b2, W3, b3):
    W1 = np.asarray(W1, dtype=np.float32)
    b1 = np.asarray(b1, dtype=np.float32)
    W2 = np.asarray(W2, dtype=np.float32)
    b2 = np.asarray(b2, dtype=np.float32)
    W3 = np.asarray(W3, dtype=np.float32)
    import ml_dtypes

    bf16 = ml_dtypes.bfloat16
    w1a = np.zeros((128, NPAIR * 128), np.float32)
    w2a = np.zeros((128, NPAIR * 128), np.float32)
    w3a = np.zeros((128, NPAIR * 128), np.float32)
    b2a = np.zeros((128, NPAIR), np.float32)
    for t in range(T):
        p, e = divmod(t, 2)
        k = p % 4
        w1a[32 * k + 5 * e : 32 * k + 5 * e + 5,
            128 * p + 64 * e : 128 * p + 64 * e + 64] = W1[t]
        w1a[32 * k + 10, 128 * p + 64 * e : 128 * p + 64 * e + 64] = b1[t]
        w2a[64 * e : 64 * e + 64, 128 * p + 64 * e : 128 * p + 64 * e + 64] = W2[t]
        # L3: pair p writes ps3 rows 2p, 2p+1 -> nonzero cols 2p+e in-block
        w3a[64 * e : 64 * e + 64, 128 * p + 2 * p + e] = W3[t][:, 0]
        b2a[64 * e : 64 * e + 64, p] = b2[t]
    return {
        "w1a": w1a.astype(bf16),
        "w2a": w2a.astype(bf16),
        "w3a": w3a.astype(bf16),
        "b2a": b2a,
    }


def pack_xt(features_core):
    """[bc, 26, 5] -> [nt, 4, 128, TILE] pair-aligned transposed layout."""
    bc = features_core.shape[0]
    nt = bc // TILE
    ff = np.asarray(features_core, np.float32).reshape(nt, TILE, T, C)
    import ml_dtypes

    xt = np.zeros((nt, 4, 128, TILE), ml_dtypes.bfloat16)
    for t in range(T):
        p, e = divmod(t, 2)
        q, k = divmod(p, 4)
        xt[:, q, 32 * k + 5 * e : 32 * k + 5 * e + 5, :] = ff[:, :, t, :].swapaxes(
            1, 2
        )
    for p in range(NPAIR):
        q, k = divmod(p, 4)
        xt[:, q, 32 * k + 2 * C, :] = 1.0
    return xt


def kernel(features, W1, b1, W2, b2, W3, b3):
    global LAST_RESULTS
    features = np.asarray(features, dtype=np.float32)
    b3 = np.asarray(b3, dtype=np.float32)
    ins = pack_weights(W1, b1, W2, b2, W3, b3)
    nc = build_nc(BC)
    nc.finalize()
    in_maps = []
    for c in range(NCORES):
        m = dict(ins)
        m["xt"] = pack_xt(features[c * BC : (c + 1) * BC])
        in_maps.append(m)
    trace = bool(int(os.environ.get("KERNEL_TRACE", "0")))
    # The first execution of a freshly loaded NEFF intermittently faults with
    # NRT_EXEC_UNIT_UNRECOVERABLE; a retry on the recovered device succeeds.
    last_exc = None
    for attempt in range(3):
        try:
            res = run_bass_kernel_spmd(
                nc, in_maps, list(range(NCORES)), trace=trace
            )
            LAST_RESULTS = res
            # out is type-major [26, bc]; transpose + b3 on the host
            return np.concatenate(
                [
                    res.results[c]["out"].T + b3[:, 0][None, :]
                    for c in range(NCORES)
                ],
                axis=0,
            )
        except Exception as e:  # noqa: BLE001
            last_exc = e
            import time as _time

            _time.sleep(5.0 * (attempt + 1))
    raise last_exc

